# revision 1
# baseline (speedup 1.0000x reference)
"""Trainium2 Bass kernel for nn_DynamiSE (GNN message passing + RK4 ODE).

Self-contained: host-side graph preprocessing (degree-sorted rounds,
4-node-packed fp16 gather tables, int16 wrapped indices, select masks) +
an SPMD Bass/Tile program for 8 NeuronCores (pos ODE on cores 0-3, neg on
4-7; AllGather exchange per RK4 stage; dma_gather for neighbor gathers;
DVE block-transpose + blockdiag matmuls for the tiny GCN weight matmuls).

Entry point: kernel(**inputs) -> np.ndarray [100000, 32] float32.
"""

import numpy as np

import numpy as np

SEG_COLS = 100          # max columns per gather segment (12800 idx < 15360)


def degree_sort(dst_local, nreal, npos):
    """positions: sort nodes by in-degree descending (stable). Returns
    pos_of_node [nreal] and deg [nreal]."""
    deg = np.bincount(dst_local, minlength=nreal)
    order = np.argsort(-deg, kind="stable")
    pos_of = np.empty(nreal, np.int64)
    pos_of[order] = np.arange(nreal)
    return pos_of, deg


def build_rounds(src, dst_local, pos_of, deg, nreal, npos, zero_row,
                 row_of_src, sub_of_src):
    """Build the slot stream for one core.

    Returns dict with:
      cols_r: list of column counts per round (before global max-merge)
      stream_rows / stream_subs: per round r, arrays [cols_r*128] of table
        rows / sub-slots (zero_row & sub 0 for padding slots)
    """
    # order edges by (position of dst, then arbitrary); rank within dst = round
    p = pos_of[dst_local]
    order = np.lexsort((np.arange(len(src)), p))
    ps, ss = p[order], src[order]
    # rank within each dst: positions sorted, so ranks via running count
    # ps is sorted; rank = index - first occurrence index of ps value
    first = np.searchsorted(ps, ps)
    rank = np.arange(len(ps)) - first
    rounds = []
    maxdeg = int(deg.max()) if len(deg) else 0
    for r in range(maxdeg):
        sel = rank == r
        psr, ssr = ps[sel], ss[sel]          # already sorted by position
        n_r = len(psr)                        # == count(deg > r)
        assert n_r == int((deg > r).sum())
        assert np.all(psr == np.arange(n_r))  # prefix property
        cols = (n_r + 127) // 128
        rows = np.full(cols * 128, zero_row, np.int64)
        subs = np.zeros(cols * 128, np.int64)
        rows[:n_r] = row_of_src[ssr]
        subs[:n_r] = sub_of_src[ssr]
        rounds.append((cols, rows, subs))
    return rounds


def pad_rounds_to(rounds, cols_r_glob, zero_row):
    """Pad this core's rounds to the global per-round column counts."""
    out = []
    for r, cols in enumerate(cols_r_glob):
        if r < len(rounds):
            c0, rows, subs = rounds[r]
        else:
            c0, rows, subs = 0, np.empty(0, np.int64), np.empty(0, np.int64)
        if c0 < cols:
            rows = np.concatenate(
                [rows, np.full((cols - c0) * 128, zero_row, np.int64)])
            subs = np.concatenate([subs, np.zeros((cols - c0) * 128, np.int64)])
        out.append((rows, subs))
    return out


def make_segments(cols_r_glob, seg_cols=SEG_COLS):
    """Split the round streams into segments.

    Returns list of segments; each is dict:
      windows: list of (round, acc_col_start(=0), width_cols, seg_col_off)
      ncols: total columns in segment
    Big rounds are split into multiple single-window segments; consecutive
    small rounds are merged into one segment with multiple windows.
    """
    segs = []
    pend_windows, pend_cols = [], 0

    def flush():
        nonlocal pend_windows, pend_cols
        if pend_windows:
            segs.append({"windows": pend_windows, "ncols": pend_cols})
            pend_windows, pend_cols = [], 0

    for r, cols in enumerate(cols_r_glob):
        if cols >= seg_cols // 2:
            flush()
            a = 0
            while a < cols:
                w = min(seg_cols, cols - a)
                segs.append({"windows": [(r, a, w, 0)], "ncols": w})
                a += w
        else:
            if pend_cols + cols > seg_cols:
                flush()
            pend_windows.append((r, 0, cols, pend_cols))
            pend_cols += cols
    flush()
    return segs


def wrap_indices(rows_stream):
    """int16-wrap a flat index array [n] (n % 128 == 0) into the dma_gather
    layout [128, n//16]: index i -> (i%16, i//16), replicated to 8 groups."""
    n = len(rows_stream)
    assert n % 128 == 0
    w = np.zeros((16, n // 16), np.int16)
    idxs = np.arange(n)
    w[idxs % 16, idxs // 16] = rows_stream.astype(np.int16)
    return np.tile(w, (8, 1))


def build_system(src, dst, owner_of, local_of, nshards, nreal_shard, npos,
                 seg_cols=SEG_COLS):
    """Build full gather systems for all `nshards` destination shards of one
    graph (one edge list).

    src/dst: global node ids [E]; owner_of/local_of: arrays [N] giving the
    OWNER SHARD and local node id of each node in THIS system's node
    partition (the same partition used for both src tables and dst shards).

    Returns per-core dicts + global segment structure. Table row of node n:
    owner_of[n]*(npos//4) + sigma_owner(n)//4.  (sigma computed here.)
    """
    N = len(owner_of)
    grp_per_shard = npos // 4

    # per-shard degree sort (over this system's edges)
    pos_of = np.empty(N, np.int64)
    deg_sh = []
    for c in range(nshards):
        nodes = np.where(owner_of == c)[0]
        dmask = owner_of[dst] == c
        dl = local_of[dst[dmask]]
        deg = np.bincount(dl, minlength=nreal_shard)
        order = np.argsort(-deg, kind="stable")
        po = np.empty(nreal_shard, np.int64)
        po[order] = np.arange(nreal_shard)
        pos_of[nodes] = po[local_of[nodes]]
        deg_sh.append(deg)

    row_of = owner_of * grp_per_shard + pos_of // 4
    sub_of = pos_of % 4

    # rounds per shard
    all_rounds, ncols_by_shard = [], []
    for c in range(nshards):
        dmask = owner_of[dst] == c
        s_c, d_c = src[dmask], dst[dmask]
        dl = local_of[d_c]
        zero_row = c * grp_per_shard + (npos // 4 - 1)  # dead group of shard c
        rounds = build_rounds(s_c, dl, pos_of_shard(pos_of, local_of, owner_of, c, nreal_shard),
                              deg_sh[c], nreal_shard, npos, zero_row,
                              row_of, sub_of)
        all_rounds.append(rounds)
        ncols_by_shard.append([r[0] for r in rounds])

    nr = max(len(x) for x in ncols_by_shard)
    cols_r_glob = [max((x[r] if r < len(x) else 0) for x in ncols_by_shard)
                   for r in range(nr)]
    segs = make_segments(cols_r_glob, seg_cols)
    totcols = sum(s["ncols"] for s in segs)

    cores = []
    for c in range(nshards):
        zero_row = c * grp_per_shard + (npos // 4 - 1)
        padded = pad_rounds_to(all_rounds[c], cols_r_glob, zero_row)
        rows_stream = np.empty(totcols * 128, np.int64)
        subs_stream = np.empty(totcols * 128, np.int64)
        off = 0
        for s in segs:
            for (r, a, w, so) in s["windows"]:
                rows, subs = padded[r]
                rows_stream[off + so * 128:(off + so * 128) + w * 128] = \
                    rows[a * 128:(a + w) * 128]
                subs_stream[off + so * 128:(off + so * 128) + w * 128] = \
                    subs[a * 128:(a + w) * 128]
            off += s["ncols"] * 128
        gidx = wrap_indices(rows_stream)
        # masks [128, totcols, 4] fp16
        masks = np.zeros((128, totcols, 4), np.float16)
        ii = np.arange(totcols * 128)
        masks[ii % 128, ii // 128, subs_stream] = 1.0
        cores.append({"gidx": gidx, "masks": masks,
                      "pos_of": None, "deg": deg_sh[c]})
    sysd = {"segs": segs, "totcols": totcols, "cols_r": cols_r_glob,
            "cores": cores, "pos_of": pos_of, "row_of": row_of,
            "sub_of": sub_of}
    return sysd


def pos_of_shard(pos_of, local_of, owner_of, c, nreal_shard):
    """per-shard position lookup indexed by LOCAL node id."""
    res = np.empty(nreal_shard, np.int64)
    nodes = np.where(owner_of == c)[0]
    res[local_of[nodes]] = pos_of[nodes]
    return res


def perm_gather_system(target_nodes, row_of, sub_of, npos_t, zero_row):
    """A plain permutation gather (no rounds): slot i <- table row of
    target_nodes[i]; pads (i >= len) -> zero_row. npos_t = padded slot count
    (multiple of 128). Returns gidx [128, npos_t//16] i16, masks
    [128, npos_t//128, 4] f16."""
    n = len(target_nodes)
    rows = np.full(npos_t, zero_row, np.int64)
    subs = np.zeros(npos_t, np.int64)
    rows[:n] = row_of[target_nodes]
    subs[:n] = sub_of[target_nodes]
    gidx = wrap_indices(rows)
    cols = npos_t // 128
    masks = np.zeros((128, cols, 4), np.float16)
    ii = np.arange(npos_t)
    masks[ii % 128, ii // 128, subs] = 1.0
    return gidx, masks

import numpy as np

H_ODE = 0.1


def shard_meta(N, nshards, npos):
    """canonical contiguous shards; returns owner_of[N], local_of[N]."""
    per = N // nshards
    owner = np.minimum(np.arange(N) // per, nshards - 1)
    local = np.arange(N) - owner * per
    return owner, local, per


def degree_sort_positions(dst, owner_of, local_of, nshards, per, npos):
    """per-shard in-degree sort; returns pos_of[N] (position within owner's
    shard), deg[N] (in-degree of each node in this edge list)."""
    N = len(owner_of)
    deg_all = np.bincount(dst, minlength=N)
    pos_of = np.empty(N, np.int64)
    for c in range(nshards):
        nodes = np.where(owner_of == c)[0]
        order = nodes[np.argsort(-deg_all[nodes], kind="stable")]
        pos_of[order] = np.arange(len(nodes))
    return pos_of, deg_all


def build_streams(src, dst, owner_of, pos_of, nshards, npos, row_of, sub_of,
                  seg_cols, coef=None):
    """Round/segment streams for one edge list over `nshards` dst shards.

    Returns (segs, totcols, per_shard list of (gidx, masks)).
    """
    grp = npos // 4
    # per-shard rounds
    if coef is None:
        coef = np.ones(len(src), np.float32)
    shard_rounds = []
    for c in range(nshards):
        m = owner_of[dst] == c
        s_c = src[m]
        cf_c = coef[m]
        p_c = pos_of[dst[m]]
        order = np.lexsort((np.arange(len(s_c)), p_c))
        ps, ss, cs = p_c[order], s_c[order], cf_c[order]
        first = np.searchsorted(ps, ps)
        rank = np.arange(len(ps)) - first
        maxdeg = int(rank.max()) + 1 if len(rank) else 0
        rounds = []
        for r in range(maxdeg):
            sel = rank == r
            psr, ssr, csr = ps[sel], ss[sel], cs[sel]
            assert np.array_equal(psr, np.arange(len(psr)))
            rounds.append((len(psr), ssr, csr))
        shard_rounds.append(rounds)

    nr = max(len(r) for r in shard_rounds)
    cols_r = [max((len(sr[r][1]) if r < len(sr) else 0)
                  for sr in shard_rounds) for r in range(nr)]
    cols_r = [(n + 127) // 128 for n in cols_r]
    segs = make_segments(cols_r, seg_cols)
    totcols = sum(s["ncols"] for s in segs)

    per_shard = []
    for c in range(nshards):
        zero_row = c * grp + (grp - 1)
        rows_stream = np.full(totcols * 128, zero_row, np.int64)
        subs_stream = np.zeros(totcols * 128, np.int64)
        vals_stream = np.zeros(totcols * 128, np.float32)
        off = 0
        for s in segs:
            for (r, a, w, so) in s["windows"]:
                if r < len(shard_rounds[c]):
                    _, ssr, csr = shard_rounds[c][r]
                    lo, hi = a * 128, min((a + w) * 128, len(ssr))
                    if hi > lo:
                        dstslice = slice(off + so * 128,
                                         off + so * 128 + (hi - lo))
                        rows_stream[dstslice] = row_of[ssr[lo:hi]]
                        subs_stream[dstslice] = sub_of[ssr[lo:hi]]
                        vals_stream[dstslice] = csr[lo:hi]
            off += s["ncols"] * 128
        gidx = wrap_indices(rows_stream)
        masks = np.zeros((128, totcols, 4), np.float16)
        ii = np.arange(totcols * 128)
        masks[ii % 128, ii // 128, subs_stream] = vals_stream
        per_shard.append((gidx, masks))
    return segs, totcols, per_shard


def blockdiag4(W):
    """[128, 4*Wout] lhsT tiles: list over 32-row chunks of W.
    wtile_k[32*bi+f, 32*bi+o] = W[32k+f, o]."""
    fin, fout = W.shape
    assert fout == 32 and fin % 32 == 0
    tiles = []
    for k in range(fin // 32):
        t = np.zeros((128, 128), np.float32)
        for bi in range(4):
            t[32 * bi:32 * bi + 32, 32 * bi:32 * bi + 32] = \
                W[32 * k:32 * k + 32, :]
        tiles.append(t)
    return np.stack(tiles)


def build_all(inputs, seg_cols=80, nstep=10):
    N = inputs["H_t"].shape[0]
    NC = 8
    perA = N // 8
    perB = N // 4
    # >=4 dead positions per shard (zero group lives there)
    CA = (perA + 4 + 127) // 128
    CB = (perB + 4 + 127) // 128
    APOS, OPOS = CA * 128, CB * 128

    edges_all = np.concatenate([inputs["A_pos_t"], inputs["A_neg_t"]],
                               axis=1).astype(np.int64)
    dp = inputs["dA_pos"].astype(np.int64)
    dn = inputs["dA_neg"].astype(np.int64)

    ownA, locA, _ = shard_meta(N, 8, APOS)
    ownB, locB, _ = shard_meta(N, 4, OPOS)

    posA, degA_all = degree_sort_positions(edges_all[1], ownA, locA, 8, perA,
                                           APOS)
    rowA = ownA * (APOS // 4) + posA // 4
    subA = posA % 4
    posP, degP_all = degree_sort_positions(dp[1], ownB, locB, 4, perB, OPOS)
    rowP = ownB * (OPOS // 4) + posP // 4
    subP = posP % 4
    posN, degN_all = degree_sort_positions(dn[1], ownB, locB, 4, perB, OPOS)
    rowN = ownB * (OPOS // 4) + posN // 4
    subN = posN % 4

    dinvA_all = 1.0 / np.sqrt(1.0 + degA_all)
    dinvP_all = 1.0 / np.sqrt(1.0 + degP_all)
    dinvN_all = 1.0 / np.sqrt(1.0 + degN_all)
    coefA = (dinvA_all[edges_all[0]] * dinvA_all[edges_all[1]]).astype(
        np.float32)
    coefP = (dinvP_all[dp[0]] * dinvP_all[dp[1]]).astype(np.float32)
    coefN = (dinvN_all[dn[0]] * dinvN_all[dn[1]]).astype(np.float32)
    segA, TA, shA = build_streams(edges_all[0], edges_all[1], ownA, posA, 8,
                                  APOS, rowA, subA, seg_cols, coef=coefA)
    segP, TP, shP = build_streams(dp[0], dp[1], ownB, posP, 4, OPOS, rowP,
                                  subP, seg_cols, coef=coefP)
    segN, TN, shN = build_streams(dn[0], dn[1], ownB, posN, 4, OPOS, rowN,
                                  subN, seg_cols, coef=coefN)
    # unify pos/neg stream shapes into one segB structure (same program!)
    colsP = round_cols(segP)
    colsN = round_cols(segN)
    nr = max(len(colsP), len(colsN))
    cols = [max(colsP[r] if r < len(colsP) else 0,
                colsN[r] if r < len(colsN) else 0) for r in range(nr)]
    segB = make_segments(cols, seg_cols)
    segB, TB, shP = rebuild_streams(dp, ownB, posP, 4, OPOS, rowP,
                                    subP, segB, coef=coefP)
    _, _, shN = rebuild_streams(dn, ownB, posN, 4, OPOS, rowN,
                                subN, segB, coef=coefN)

    # H redistribution and phase-C gather systems
    node_at_posB = np.full((4, OPOS), -1, np.int64)
    node_at_posB[ownB, posP] = np.arange(N)
    node_at_posB_n = np.full((4, OPOS), -1, np.int64)
    node_at_posB_n[ownB, posN] = np.arange(N)

    segH = make_segments([CB], seg_cols)
    TH = sum(s["ncols"] for s in segH)
    segC = make_segments([CA], seg_cols)
    TC = sum(s["ncols"] for s in segC)
    assert TH == CB and TC == CA

    # per-core inputs
    W_init = np.asarray(inputs["W_init"], np.float32)
    wib = blockdiag4(W_init)
    wblk_p = blockdiag4(np.asarray(inputs["W_pos"], np.float32))[0]
    wblk_n = blockdiag4(np.asarray(inputs["W_neg"], np.float32))[0]
    cwb = blockdiag4(np.asarray(inputs["W_comb"], np.float32))
    bi32 = np.tile(np.asarray(inputs["b_init"], np.float32), (128, 1))
    lng = np.tile(np.asarray(inputs["ln_g"], np.float32), (128, 1))
    lnb = np.tile(np.asarray(inputs["ln_b"], np.float32), (128, 1))
    cbstk = np.tile(np.asarray(inputs["b_comb"], np.float32), 4)[:, None]
    bstk_p = np.tile(np.asarray(inputs["b_pos"], np.float32), 4)[:, None]
    bstk_n = np.tile(np.asarray(inputs["b_neg"], np.float32), 4)[:, None]
    wt_p = np.tile(np.asarray(inputs["wt_pos"], np.float32), 4)[:, None]
    wt_n = np.tile(np.asarray(inputs["wt_neg"], np.float32), 4)[:, None]
    offs = np.array([0.0, 0.5, 0.5, 1.0]) * H_ODE
    tg = (np.arange(nstep)[:, None] * H_ODE + offs[None, :]).reshape(-1)
    tgrid = np.tile(tg.astype(np.float32), (128, 1))

    Ht = np.asarray(inputs["H_t"], np.float32)

    def pos_pack(vals, owner, pos, c, npos, fill=0.0):
        """vals[N] -> [128, npos//128, 1] f32 position-space for shard c."""
        arr = np.full(npos, fill, np.float32)
        nodes = np.where(owner == c)[0]
        arr[pos[nodes]] = vals[nodes]
        return arr.reshape(npos // 128, 128, 1).transpose(1, 0, 2).copy()

    in_maps = []
    for c in range(NC):
        grp = c // 4          # 0 = pos, 1 = neg
        cb = c % 4
        nodesA = np.where(ownA == c)[0]
        ordA = nodesA[np.argsort(posA[nodesA])]
        ht_c = np.zeros((APOS, 128), np.float32)
        ht_c[:len(ordA)] = Ht[ordA]
        assert np.array_equal(posA[ordA], np.arange(len(ordA)))

        degA_c = pos_pack(dinvA_all * dinvA_all, ownA, posA, c, APOS,
                          fill=0.0)
        if grp == 0:
            deg_c = pos_pack(dinvP_all * dinvP_all, ownB, posP, cb, OPOS,
                             fill=0.0)
            gxB_c, mB_c = shP[cb]
            wblk_c, bstk_c, wt_c = wblk_p, bstk_p, wt_p
            nap = node_at_posB[cb]
        else:
            deg_c = pos_pack(dinvN_all * dinvN_all, ownB, posN, cb, OPOS,
                             fill=0.0)
            gxB_c, mB_c = shN[cb]
            wblk_c, bstk_c, wt_c = wblk_n, bstk_n, wt_n
            nap = node_at_posB_n[cb]

        # H redistribution: slot q -> node at ODE position q of my shard
        rowsH = np.zeros(OPOS, np.int64)
        subsH = np.zeros(OPOS, np.int64)
        valid = nap >= 0
        rowsH[valid] = rowA[nap[valid]]
        subsH[valid] = subA[nap[valid]]
        gxH_c = wrap_indices(rowsH)
        mH_c = np.zeros((128, CB, 4), np.float16)
        ii = np.arange(OPOS)
        mH_c[ii % 128, ii // 128, subsH] = 1.0

        # phase C: canonical nodes of 8-way shard c
        n0 = c * perA
        tgt = np.arange(n0, n0 + perA)
        rowsCp = np.zeros(APOS, np.int64)
        subsCp = np.zeros(APOS, np.int64)
        rowsCp[:perA] = rowP[tgt]
        subsCp[:perA] = subP[tgt]
        rowsCn = np.zeros(APOS, np.int64)
        subsCn = np.zeros(APOS, np.int64)
        rowsCn[:perA] = rowN[tgt]
        subsCn[:perA] = subN[tgt]
        gxCp_c = wrap_indices(rowsCp)
        gxCn_c = wrap_indices(rowsCn)
        mCp_c = np.zeros((128, CA, 4), np.float16)
        mCn_c = np.zeros((128, CA, 4), np.float16)
        jj = np.arange(APOS)
        mCp_c[jj % 128, jj // 128, subsCp] = 1.0
        mCn_c[jj % 128, jj // 128, subsCn] = 1.0

        in_maps.append({
            "ht": ht_c, "degA": degA_c, "degB": deg_c,
            "wib": wib, "wblk": wblk_c, "cwb": cwb,
            "bi32": bi32, "bstk": bstk_c, "cbstk": cbstk,
            "wtstk": wt_c, "tgrid": tgrid, "lng": lng, "lnb": lnb,
            "gxA": shA[c][0], "mA": shA[c][1],
            "gxB": gxB_c, "mB": mB_c,
            "gxH": gxH_c, "mH": mH_c,
            "gxCp": gxCp_c, "mCp": mCp_c,
            "gxCn": gxCn_c, "mCn": mCn_c,
        })

    cfg = {"CA": CA, "CB": CB, "TA": TA, "TB": TB, "TH": TH, "TC": TC,
           "segA": segA, "segB": segB, "segH": segH, "segC": segC,
           "NSTEP": nstep}
    meta = {"perA": perA, "cfg": cfg}
    return cfg, in_maps, meta


def round_cols(segs):
    """recover per-round col counts from a segment structure."""
    cols = {}
    for s in segs:
        for (r, a, w, so) in s["windows"]:
            cols[r] = max(cols.get(r, 0), a + w)
    return [cols[r] for r in sorted(cols)]


def rebuild_streams(d, ownB, pos, nshards, npos, row_of, sub_of, segs,
                    coef=None):
    """build per-shard streams for a FIXED segment structure."""
    src, dst = d[0], d[1]
    if coef is None:
        coef = np.ones(len(src), np.float32)
    grp = npos // 4
    totcols = sum(s["ncols"] for s in segs)
    per_shard = []
    for c in range(nshards):
        m = ownB[dst] == c
        s_c = src[m]
        cf_c = coef[m]
        p_c = pos[dst[m]]
        order = np.lexsort((np.arange(len(s_c)), p_c))
        ps, ss, cs = p_c[order], s_c[order], cf_c[order]
        first = np.searchsorted(ps, ps)
        rank = np.arange(len(ps)) - first
        zero_row = c * grp + (grp - 1)
        rows_stream = np.full(totcols * 128, zero_row, np.int64)
        subs_stream = np.zeros(totcols * 128, np.int64)
        vals_stream = np.zeros(totcols * 128, np.float32)
        off = 0
        for s in segs:
            for (r, a, w, so) in s["windows"]:
                sel = rank == r
                ssr = ss[sel]
                csr = cs[sel]
                lo, hi = a * 128, min((a + w) * 128, len(ssr))
                if hi > lo:
                    dsts = slice(off + so * 128, off + so * 128 + (hi - lo))
                    rows_stream[dsts] = row_of[ssr[lo:hi]]
                    subs_stream[dsts] = sub_of[ssr[lo:hi]]
                    vals_stream[dsts] = csr[lo:hi]
            off += s["ncols"] * 128
        gidx = wrap_indices(rows_stream)
        masks = np.zeros((128, totcols, 4), np.float16)
        ii = np.arange(totcols * 128)
        masks[ii % 128, ii // 128, subs_stream] = vals_stream
        per_shard.append((gidx, masks))
    return segs, totcols, per_shard


def posB_safe(p):
    return p


def assemble_output(results, perA, N):
    outs = [results[c]["o"][:perA] for c in range(8)]
    return np.concatenate(outs, axis=0)[:N]

import numpy as np
from concourse import bass, bacc, mybir
import concourse.tile as tile
from concourse import library_config

F32 = mybir.dt.float32
F16 = mybir.dt.float16
I16 = mybir.dt.int16
AL = mybir.AluOpType
ACTF = mybir.ActivationFunctionType


def pos_packed_dram_ap(t, cols, feat):
    """DRAM AP over flat tensor t (POS*feat elems) shaped [128, cols, feat]
    so that position p=(c*128+part) maps to flat offset p*feat."""
    flat = t[:, :].rearrange("a b -> (a b)")
    return flat.rearrange("(c p f) -> p c f", p=128, f=feat)


NQ = 4                  # SWDGE queues; gathers round-robin across them
_QCTR = [0]


def emit_seg_gathers(nc, pool, table_ap, idx_tile, idx_coloff, mask_tile,
                     segs, acc, gtag="g", stream=None, acc2=None,
                     no_select=False, post_seg=None):
    """Gather segments + select-add into acc [128, C, 32] (f32).

    Resident mode: idx_tile [128, K] i16 + mask_tile [128, T, 4, 1] f16.
    Streaming mode (stream=(gx_param, m_param, spool)): per-segment DMA of
    idx/mask slices from DRAM; idx_tile/mask_tile ignored.
    """
    stream_col = 0
    accs = [acc] if acc2 is None else [acc, acc2]
    nacc = len(accs)
    wi = 0
    for si, s in enumerate(segs):
        w = s["ncols"]
        g = pool.tile([128, w, 4, 32], F16, tag=gtag)
        if stream is not None:
            gx_p, m_p, spool = stream
            idx_tile = spool.tile([128, w * 8], I16, tag=gtag + "sx")
            nc.sync.dma_start(
                out=idx_tile[:, :],
                in_=gx_p[:, stream_col * 8:(stream_col + w) * 8])
            mask_tile = spool.tile([128, w, 4, 1], F16, tag=gtag + "sm")
            nc.sync.dma_start(
                out=mask_tile[:, :, :, :],
                in_=m_p[:, stream_col:stream_col + w, :, None])
            c8 = 0
            mbase = stream_col
        else:
            c8 = idx_coloff + stream_col * 8
            mbase = 0
        nc.gpsimd.dma_gather(
            out_ap=g[:, :, :, :].rearrange("p w a b -> p w (a b)"),
            in_ap=table_ap,
            idxs_ap=idx_tile[:, c8:c8 + w * 8],
            num_idxs=w * 128,
            num_idxs_reg=w * 128,
            elem_size=128,
            single_packet=False,
            queue_num=_QCTR[0] % NQ,
        )
        _QCTR[0] += 1
        if no_select:
            # perf attribution only: consume one column so the tile dep exists
            nc.vector.tensor_tensor(
                out=acc[:, 0:1, :], in0=acc[:, 0:1, :],
                in1=g[:, 0, 0, :][:, None, :], op=AL.add)
            if post_seg is not None:
                post_seg(si)
            stream_col += w
            continue
        mb0 = stream_col - mbase
        # segment-wide coef-masked select + pair tree (masks are
        # dinv_src*dinv_dst for real slots, 0 for padding)
        nc.vector.tensor_tensor(
            out=g[:, 0:w, 0:2, :], in0=g[:, 0:w, 0:2, :],
            in1=mask_tile[:, mb0:mb0 + w, 0:2, :]
                .to_broadcast([128, w, 2, 32]),
            op=AL.mult)
        nc.vector.tensor_tensor(
            out=g[:, 0:w, 2:4, :], in0=g[:, 0:w, 2:4, :],
            in1=mask_tile[:, mb0:mb0 + w, 2:4, :]
                .to_broadcast([128, w, 2, 32]),
            op=AL.mult)
        nc.vector.tensor_tensor(
            out=g[:, 0:w, 0:2, :], in0=g[:, 0:w, 0:2, :],
            in1=g[:, 0:w, 2:4, :], op=AL.add)
        nc.vector.tensor_tensor(
            out=g[:, 0:w, 0, :], in0=g[:, 0:w, 0, :],
            in1=g[:, 0:w, 1, :], op=AL.add)
        for (r, a, ww, so) in s["windows"]:
            tgt = accs[wi % nacc]
            wi += 1
            nc.vector.tensor_tensor(
                out=tgt[:, a:a + ww, :], in0=tgt[:, a:a + ww, :],
                in1=g[:, so:so + ww, 0, :], op=AL.add)
        if post_seg is not None:
            post_seg(si)
        stream_col += w


def emit_tl_matmul(nc, pool, psum_pool, src, cols, fin, wtiles, bias_tile,
                   out, act_func, gate=None, scale=1.0, ztag="zt", ytag="yt",
                   src_dram=None):
    """TL-transform matmul: out[:, :, 0:32] = act(src @ W + bias) (*gate).

    src: [128, cols, fin] f32 position layout; wtiles: list of lhsT
    [128,128] tiles (one per 32-wide fin chunk, blockdiag over the 4
    node-subgroups); bias_tile: [128,1] (per TL partition); out: [128, cols,
    32] f32; gate: optional [128,1] AP multiplied after act.
    cols*fin assumed; processes chunks of cc=512//fin columns.
    """
    cc = 512 // fin
    nk = fin // 32
    nch = (cols + cc - 1) // cc
    for j in range(nch):
        c0 = j * cc
        w = min(cc, cols - c0)
        if src_dram is not None:
            st = pool.tile([128, cc, fin], F32, tag=ztag + "ld")
            nc.sync.dma_start(out=st[:, 0:w, :],
                              in_=src_dram(c0, w))
            src_ap = st[:, 0:w, :]
        else:
            src_ap = src[:, c0:c0 + w, :]
        zt = pool.tile([128, 512], F32, tag=ztag)
        nc.vector.transpose(out=zt[:, 0:w * fin],
                            in_=src_ap.rearrange("p c f -> p (c f)"))
        ps = psum_pool.tile([128, cc * 32], F32, space="PSUM", tag="ps")
        for k in range(nk):
            rhs = zt[:, 0:w * fin].rearrange("p (c k f) -> p c k f", k=nk,
                                             f=32)[:, :, k, :]
            nc.tensor.matmul(out=ps[:, 0:w * 32], lhsT=wtiles[k][:, :],
                             rhs=rhs, start=(k == 0), stop=(k == nk - 1))
        yt = pool.tile([128, cc * 32], F32, tag=ytag)
        nc.scalar.activation(out=yt[:, 0:w * 32], in_=ps[:, 0:w * 32],
                             func=act_func, bias=bias_tile, scale=scale)
        if gate is not None:
            nc.vector.tensor_scalar_mul(yt[:, 0:w * 32], yt[:, 0:w * 32],
                                        gate)
        nc.vector.transpose(out=out[:, c0:c0 + w, :]
                            .rearrange("p c f -> p (c f)"),
                            in_=yt[:, 0:w * 32])


def build_program(cfg, ablate=()):
    """cfg: dict with sizes + host-built systems (see gnn_kernel.py).

    ablate: set of feature names to SKIP (perf attribution only — results
    become wrong): "collB", "gatherB", "mmB", "phaseA", "phaseC", "vecB".
    """
    ab = set(ablate)
    CA = cfg["CA"]            # phase-A columns (APOS//128)
    CB = cfg["CB"]            # ODE columns
    NC = 8
    APOS, OPOS = CA * 128, CB * 128
    AGRP, OGRP = APOS // 4, OPOS // 4
    TA, TB = cfg["TA"], cfg["TB"]      # stream columns of A and B systems
    segA, segB = cfg["segA"], cfg["segB"]
    segH, segC = cfg["segH"], cfg["segC"]
    TH, TC = cfg["TH"], cfg["TC"]
    NSTEP = cfg["NSTEP"]
    H_ODE = 0.1

    nc = bacc.Bacc("TRN2", target_bir_lowering=False, debug=False,
                   num_devices=NC, num_swdge_queues=NQ)
    _QCTR[0] = 0

    def param(name, shape, dt=F32, out=False):
        return nc.declare_dram_parameter(name, list(shape), dt, isOutput=out)

    ht = param("ht", [APOS, 128])
    degA = param("degA", [128, CA, 1])     # dinv^2 (dead positions = 0)
    degB = param("degB", [128, CB, 1])
    wib = param("wib", [4, 128, 128])
    wblk = param("wblk", [128, 128])
    cwb = param("cwb", [2, 128, 128])
    bi32 = param("bi32", [128, 32])
    bstk = param("bstk", [128, 1])
    cbstk = param("cbstk", [128, 1])
    wtstk = param("wtstk", [128, 1])
    tgrid = param("tgrid", [128, 4 * NSTEP])
    lng = param("lng", [128, 32])
    lnb = param("lnb", [128, 32])
    gxA = param("gxA", [128, TA * 8], I16)
    mA = param("mA", [128, TA, 4], F16)
    gxB = param("gxB", [128, TB * 8], I16)
    mB = param("mB", [128, TB, 4], F16)
    gxH = param("gxH", [128, TH * 8], I16)
    mH = param("mH", [128, TH, 4], F16)
    gxCp = param("gxCp", [128, TC * 8], I16)
    mCp = param("mCp", [128, TC, 4], F16)
    gxCn = param("gxCn", [128, TC * 8], I16)
    mCn = param("mCn", [128, TC, 4], F16)
    out_o = param("o", [APOS, 32], out=True)

    aginA = nc.dram_tensor("aginA", [AGRP, 128], F16)
    tableA = nc.dram_tensor("tableA", [8 * AGRP, 128], F16,
                            addr_space="Shared")
    aginH = nc.dram_tensor("aginH", [AGRP, 128], F16)
    tableH = nc.dram_tensor("tableH", [8 * AGRP, 128], F16,
                            addr_space="Shared")
    aginB = nc.dram_tensor("aginB", [OGRP, 128], F16)
    # 4-rank groups cannot use a Shared-space output; Local works (slower
    # ncfw path, acceptable)
    tableB = nc.dram_tensor("tableB", [4 * OGRP, 128], F16)
    aginC = nc.dram_tensor("aginC", [OGRP, 128], F16)
    tableC = nc.dram_tensor("tableC", [8 * OGRP, 128], F16,
                            addr_space="Shared")

    RG_ALL = [list(range(NC))]
    RG_HALF = [[0, 1, 2, 3], [4, 5, 6, 7]]

    with tile.TileContext(nc) as tc:
        with (
            tc.tile_pool(name="const", bufs=1) as cp,
            tc.tile_pool(name="state", bufs=1) as sp,
            tc.tile_pool(name="work", bufs=2) as wp,
            tc.tile_pool(name="gpool", bufs=8) as gp,
            tc.tile_pool(name="psum", bufs=2, space="PSUM") as pp,
        ):
            nc.gpsimd.load_library(library_config.mlp)

            # ---- constants / small tiles ----
            wib_t = []
            for k in range(4):
                t = cp.tile([128, 128], F32, tag=f"wib{k}")
                nc.sync.dma_start(out=t[:], in_=wib[k, :, :])
                wib_t.append(t)

            wblk_t = cp.tile([128, 128], F32)
            nc.sync.dma_start(out=wblk_t[:], in_=wblk[:, :])

            bi32_t = cp.tile([128, 1, 32], F32)
            nc.sync.dma_start(out=bi32_t[:], in_=bi32[:, None, :])
            bstk_t = cp.tile([128, 1], F32)
            nc.sync.dma_start(out=bstk_t[:], in_=bstk[:, :])
            cbstk_t = cp.tile([128, 1], F32)
            nc.sync.dma_start(out=cbstk_t[:], in_=cbstk[:, :])
            lng_t = cp.tile([128, 1, 32], F32)
            nc.sync.dma_start(out=lng_t[:], in_=lng[:, None, :])
            lnb_t = cp.tile([128, 1, 32], F32)
            nc.sync.dma_start(out=lnb_t[:], in_=lnb[:, None, :])

            # gates[q, 4i+s] = sigmoid(t * wt)
            wtstk_t = cp.tile([128, 1], F32)
            nc.sync.dma_start(out=wtstk_t[:], in_=wtstk[:, :])
            tg_t = cp.tile([128, 4 * NSTEP], F32)
            nc.sync.dma_start(out=tg_t[:], in_=tgrid[:, :])
            gates = cp.tile([128, 4 * NSTEP], F32)
            nc.vector.tensor_scalar_mul(gates[:], tg_t[:], wtstk_t[:])
            nc.scalar.activation(out=gates[:], in_=gates[:], func=ACTF.Sigmoid)

            # dinv^2 tables (host-computed; dead positions zero)
            dinvA2 = cp.tile([128, CA, 1], F32)
            nc.sync.dma_start(out=dinvA2[:], in_=degA[:, :, :])
            dinvB2 = cp.tile([128, CB, 1], F32)
            nc.sync.dma_start(out=dinvB2[:], in_=degB[:, :, :])


            # ================= PHASE A =================
            # state tiles are declared up front; phase A borrows them:
            #   xs1 -> ksum[:, 0:CA]   accA -> acc[:, 0:CA]
            #   f16 staging -> v[:, 0:CA]
            x = sp.tile([128, CB, 32], F32, tag="x")
            ksum = sp.tile([128, CB, 32], F32, tag="ksum")
            acc = sp.tile([128, CB, 32], F32, tag="acc")
            v = sp.tile([128, CB, 32], F16, tag="v")

            if "phaseA" in ab:
                nc.vector.memset(x[:, :, :], 0.0)
                nc.vector.memset(ksum[:, :, :], 0.0)
            else:
              with tc.tile_pool(name="phas", bufs=2) as as_pool:
                ht_ap = pos_packed_dram_ap(ht, CA, 128)

                def ht_chunk(c0, w):
                    return ht_ap[:, c0:c0 + w, :]
                xs1 = ksum[:, 0:CA, :]
                emit_tl_matmul(nc, as_pool, pp, None, CA, 128, wib_t, 0.0,
                               xs1, ACTF.Identity, src_dram=ht_chunk)
                xs1h = v[:, 0:CA, :]
                nc.vector.tensor_copy(out=xs1h[:, :, :], in_=xs1[:, :, :])
                nc.sync.dma_start(out=pos_packed_dram_ap(aginA, CA, 32),
                                  in_=xs1h[:, :, :])
                nc.gpsimd.collective_compute(
                    "AllGather", AL.bypass, replica_groups=RG_ALL,
                    ins=[aginA[:, :]], outs=[tableA[:, :]])

                # self term: xs1 * dinv^2 (coef folded into gather masks)
                accA = acc[:, 0:CA, :]
                nc.vector.tensor_tensor(
                    out=accA[:, :, :], in0=xs1[:, :, :],
                    in1=dinvA2[:, :, :].to_broadcast([128, CA, 32]),
                    op=AL.mult)
                with tc.tile_pool(name="phast", bufs=4) as astr:
                    emit_seg_gathers(nc, gp, tableA[:, :], None, 0, None,
                                     segA, accA, stream=(gxA, mA, astr))
                # H = relu(accA + b_init)
                nc.vector.tensor_tensor(
                    out=accA[:, :, :], in0=accA[:, :, :],
                    in1=bi32_t[:, :, :].to_broadcast([128, CA, 32]),
                    op=AL.add)
                nc.vector.tensor_scalar(
                    out=accA[:, :, :], in0=accA[:, :, :], scalar1=0.0,
                    scalar2=None, op0=AL.max)
                accAh = v[:, 0:CA, :]
                nc.vector.tensor_copy(out=accAh[:, :, :], in_=accA[:, :, :])
                nc.sync.dma_start(out=pos_packed_dram_ap(aginH, CA, 32),
                                  in_=accAh[:, :, :])
                nc.gpsimd.collective_compute(
                    "AllGather", AL.bypass, replica_groups=RG_ALL,
                    ins=[aginH[:, :]], outs=[tableH[:, :]])

            # ================= redistribute: x0 =================
            nc.vector.memset(x[:, :, :], 0.0)
            if "phaseA" not in ab:
                with tc.tile_pool(name="phh", bufs=2) as hp:
                    emit_seg_gathers(nc, gp, tableH[:, :], None, 0, None,
                                     segH, x, stream=(gxH, mH, hp))

            # ================= PHASE B: 40 RK4 stages =================
            # chunk readiness: chunk ci (16 acc cols) is final after the
            # last segment whose windows touch cols [16ci, 16ci+16)
            NCH = (CB + 15) // 16
            last_touch = [0] * NCH
            for si, sg in enumerate(segB):
                for (r, a, w_, so) in sg["windows"]:
                    for ci in range(a // 16, min(NCH, (a + w_ + 15) // 16)):
                        last_touch[ci] = max(last_touch[ci], si)
            ready_after = [[] for _ in range(len(segB))]
            for ci, si in enumerate(last_touch):
                ready_after[si].append(ci)

            bsp_cm = tc.tile_pool(name="phbs", bufs=6)
            bsp = bsp_cm.__enter__()
            coef = [H_ODE * 0.5, H_ODE * 0.5, H_ODE]
            aginB_ap = pos_packed_dram_ap(aginB, CB, 32)
            # initial publish of x0 (v <- f16(x))
            nc.vector.tensor_copy(out=v[:, :, :], in_=x[:, :, :])
            if "collB" not in ab:
                nc.sync.dma_start(out=aginB_ap, in_=v[:, :, :])
            for i in range(NSTEP):
                for s in range(4):
                    last_sub = (i == NSTEP - 1 and s == 3)
                    if "collB" not in ab:
                        nc.gpsimd.collective_compute(
                            "AllGather", AL.bypass, replica_groups=RG_HALF,
                            ins=[aginB[:, :]], outs=[tableB[:, :]])
                    # self term: v * dinv^2 (edge coef folded into masks)
                    nc.vector.tensor_tensor(
                        out=acc[:, :, :], in0=v[:, :, :],
                        in1=dinvB2[:, :, :].to_broadcast([128, CB, 32]),
                        op=AL.mult)
                    gsc = gates[:, 4 * i + s:4 * i + s + 1]
                    gb = wp.tile([128, 1], F32, tag="gb")
                    nc.vector.tensor_tensor(out=gb[:, :], in0=bstk_t[:, :],
                                            in1=gsc, op=AL.mult)

                    def finalize_chunk(ci, i=i, s=s, gsc=gsc, gb=gb,
                                       last_sub=last_sub):
                        c0 = 16 * ci
                        w_ = min(16, CB - c0)
                        kc = acc[:, c0:c0 + w_, :]
                        if "mmB" not in ab:
                            emit_tl_matmul(
                                nc, wp, pp, acc[:, c0:c0 + w_, :], w_, 32,
                                [wblk_t], gb[:, :], kc,
                                ACTF.Relu, scale=gsc)
                        kca = kc
                        if s == 0:
                            nc.vector.tensor_copy(
                                out=ksum[:, c0:c0 + w_, :], in_=kca)
                        elif s in (1, 2):
                            nc.vector.scalar_tensor_tensor(
                                out=ksum[:, c0:c0 + w_, :], in0=kca,
                                scalar=2.0, in1=ksum[:, c0:c0 + w_, :],
                                op0=AL.mult, op1=AL.add)
                        else:
                            nc.vector.tensor_tensor(
                                out=ksum[:, c0:c0 + w_, :],
                                in0=ksum[:, c0:c0 + w_, :],
                                in1=kca, op=AL.add)
                        if s < 3:
                            nc.vector.scalar_tensor_tensor(
                                out=v[:, c0:c0 + w_, :], in0=kca,
                                scalar=float(coef[s]),
                                in1=x[:, c0:c0 + w_, :],
                                op0=AL.mult, op1=AL.add)
                        else:
                            nc.vector.scalar_tensor_tensor(
                                out=x[:, c0:c0 + w_, :],
                                in0=ksum[:, c0:c0 + w_, :],
                                scalar=H_ODE / 6.0, in1=x[:, c0:c0 + w_, :],
                                op0=AL.mult, op1=AL.add)
                            nc.vector.tensor_copy(
                                out=v[:, c0:c0 + w_, :],
                                in_=x[:, c0:c0 + w_, :])
                        if not last_sub and "collB" not in ab:
                            nc.sync.dma_start(
                                out=aginB_ap[:, c0:c0 + w_, :],
                                in_=v[:, c0:c0 + w_, :])

                    def post_seg(si):
                        for ci in ready_after[si]:
                            finalize_chunk(ci)

                    if "gatherB" not in ab:
                        emit_seg_gathers(nc, gp, tableB[:, :], None, 0, None,
                                         segB, acc, stream=(gxB, mB, bsp),
                                         no_select=("selB" in ab),
                                         post_seg=post_seg)
                    else:
                        for ci in range(NCH):
                            finalize_chunk(ci)
            bsp_cm.__exit__(None, None, None)

            # ================= PHASE C =================
            if "phaseC" not in ab:
                # Z (=x) -> fp16 table (every core publishes its ODE result)
                nc.vector.tensor_copy(out=v[:, :, :], in_=x[:, :, :])
                nc.sync.dma_start(out=pos_packed_dram_ap(aginC, CB, 32),
                                  in_=v[:, :, :])
                nc.gpsimd.collective_compute(
                    "AllGather", AL.bypass, replica_groups=RG_ALL,
                    ins=[aginC[:, :]], outs=[tableC[:, :]])
                with tc.tile_pool(name="phc", bufs=1) as cpp:
                    cwb_t = []
                    for k in range(2):
                        t = cpp.tile([128, 128], F32, tag=f"cwb{k}")
                        nc.sync.dma_start(out=t[:], in_=cwb[k, :, :])
                        cwb_t.append(t)
                    # x (the published Z) is dead after the AG; reuse its space
                    if CB >= 2 * TC:
                        z2 = x[:, 0:2 * TC, :].rearrange(
                            "p (a b) f -> p a (b f)", b=2)
                    else:
                        z2t = cpp.tile([128, TC, 64], F32)
                        z2 = z2t[:, :, :]
                    nc.vector.memset(z2[:, :, :], 0.0)
                    with tc.tile_pool(name="phcs", bufs=2) as csp:
                        emit_seg_gathers(nc, gp, tableC[0:4 * OGRP, :], None, 0,
                                         None, segC, z2[:, :, 0:32],
                                         stream=(gxCp, mCp, csp))
                        emit_seg_gathers(nc, gp, tableC[4 * OGRP:8 * OGRP, :],
                                         None, 0, None, segC, z2[:, :, 32:64],
                                         stream=(gxCn, mCn, csp))
                    zc = ksum[:, 0:TC, :] if CB >= TC else \
                        cpp.tile([128, TC, 32], F32, tag="zc")[:, :, :]
                    emit_tl_matmul(nc, wp, pp, z2, TC, 64, cwb_t,
                                   cbstk_t[:, :], zc, ACTF.Identity)
                    # layernorm over the 32 features
                    s1 = cpp.tile([128, TC, 1], F32, tag="s1")
                    nc.vector.tensor_reduce(out=s1[:, :, :], in_=zc[:, :, :],
                                            axis=mybir.AxisListType.X, op=AL.add)
                    nc.vector.tensor_scalar_mul(s1[:, :, :], s1[:, :, :],
                                                -1.0 / 32.0)
                    nc.vector.tensor_tensor(
                        out=zc[:, :, :], in0=zc[:, :, :],
                        in1=s1[:, :, :].to_broadcast([128, TC, 32]), op=AL.add)
                    sq = acc[:, 0:TC, :] if CB >= TC else \
                        cpp.tile([128, TC, 32], F32, tag="sq")[:, :, :]
                    nc.scalar.activation(out=sq[:, :, :], in_=zc[:, :, :],
                                         func=ACTF.Square)
                    v1 = cpp.tile([128, TC, 1], F32, tag="v1")
                    nc.vector.tensor_reduce(out=v1[:, :, :], in_=sq[:, :, :],
                                            axis=mybir.AxisListType.X, op=AL.add)
                    nc.vector.tensor_scalar(
                        out=v1[:, :, :], in0=v1[:, :, :], scalar1=1.0 / 32.0,
                        scalar2=1e-5, op0=AL.mult, op1=AL.add)
                    nc.scalar.activation(out=v1[:, :, :], in_=v1[:, :, :],
                                         func=ACTF.Sqrt)
                    nc.vector.reciprocal(v1[:, :, :], v1[:, :, :])
                    nc.vector.tensor_tensor(
                        out=zc[:, :, :], in0=zc[:, :, :],
                        in1=v1[:, :, :].to_broadcast([128, TC, 32]), op=AL.mult)
                    nc.vector.tensor_tensor(
                        out=zc[:, :, :], in0=zc[:, :, :],
                        in1=lng_t[:, :, :].to_broadcast([128, TC, 32]),
                        op=AL.mult)
                    nc.vector.tensor_tensor(
                        out=zc[:, :, :], in0=zc[:, :, :],
                        in1=lnb_t[:, :, :].to_broadcast([128, TC, 32]),
                        op=AL.add)
                    nc.sync.dma_start(out=pos_packed_dram_ap(out_o, TC, 32),
                                      in_=zc[:, :, :])
    nc.compile()
    return nc


# ============================ entry point ============================
_CACHE = {}


def kernel(**inputs):
    from concourse.bass_utils import run_bass_kernel_spmd

    cfg, in_maps, meta = build_all(inputs, seg_cols=48, nstep=10)
    key = "prog"
    if key not in _CACHE:
        _CACHE[key] = build_program(cfg)
    nc = _CACHE[key]
    br = run_bass_kernel_spmd(nc, in_maps, list(range(8)))
    return assemble_output(br.results, meta["perA"],
                           inputs["H_t"].shape[0]).astype(np.float32)

